# revision 21
# baseline (speedup 1.0000x reference)
"""Bass/Trainium2 kernel for nn_DecorrelationGradient.

Reference computation (KAPPA = 0.5):
    out = (1-k)*(gram - diag_ms) + k*(diag_ms - 1)
        = 0.5 * (X^T X / N) - 0.5          (diag terms cancel algebraically)

with X = x.reshape(N, d), N = 8*2048 = 16384, d = 768.

Strategy (data-parallel over the sample axis, 8 cores):
  - core c gets x[c] : [2048, 768] f32
  - per-core pipeline: HWDGE f32 loads (both rings, 16 single-tile DMAs:
    measured ~334-374 GB/s aggregate = the per-core HBM roofline) -> DVE
    casts -> PE accumulates the upper-triangle blocks of the partial
    Gram P_c = x_c^T x_c in PSUM (fp32)
  - PE dtype schedule: k-tiles 0-1 run as bf16 single-k matmuls (so the
    PE can start as soon as the FIRST tile lands); k-tiles 2-15 run as 7
    fp8e4m3 DoubleRow super-tiles (256-sample contraction, ~2x PE rate).
    This keeps the PE off the critical path even when the chip sits in
    the low power state (PE ~2.0 GHz instead of 2.4). fp8 quantization
    adds only ~2e-4 rel err: the Gram averages 16384 independent
    products, so per-product quantization noise shrinks ~sqrt(N).
  - junk matmuls on a zeroed tile bridge the PE from t=0 to the first
    real tile so the HAM clock-gate un-throttles before real work
  - fused scale+bias on the PSUM->SBUF copy (ACT/DVE split halves):
    t = P_c * (0.5/N) - 0.5/8, emitted in fp16 (halves store bytes,
    ~1e-4 rel err on the 8-partial sum)
  - the final super-tile's chunks run in completion order (big blocks
    first); finished row-block groups are stored immediately (3 stores
    on the idle SP ring) so stores overlap the remaining matmuls
  - each core outputs its scaled partial packed triangle [128, 2688]
    fp16; the host gather sums the 8 partials in fp32 (the affine above
    makes the sum equal 0.5*G/N - 0.5) and unpacks the symmetric matrix.
"""

import numpy as np

import concourse.bacc as bacc
import concourse.bass as bass  # noqa: F401
import concourse.tile as tile
from concourse import mybir
from concourse.bass_utils import run_bass_kernel_spmd

P = 128
D = 768
NSHARD = 2048          # samples per core
KT = NSHARD // P       # 16 k-tiles
KBF = 2                # leading k-tiles computed in bf16 (early PE start)
# k2..k15 run as 7 fp8 DoubleRow super-tiles (2 k-tiles each). A lone-k15
# finale was tried and is NOT better: it forces the (k13,k14) super to
# start only after k14's late cast, so the PE tail past the last cast
# grows from ~1.6us to ~2.1us.
NSUPER = (KT - KBF) // 2
NB = D // P            # 6 row/col blocks
NCORES = 8
NTOT = 8 * 2048
SCALE = 0.5 / NTOT     # 2**-15, exact
BIAS = -0.5 / NCORES   # -0.0625, exact in fp16; host sum of 8 -> -0.5

# packed upper-triangle blocks (i, j) with j >= i, row-major in i
TRI_BLOCKS = [(i, j) for i in range(NB) for j in range(i, NB)]
NTRI = len(TRI_BLOCKS)          # 21
TRI_W = NTRI * P                # 2688 packed columns

# packed column range of row-block i
OFFS = []
_o = 0
for _i in range(NB):
    OFFS.append((_o, _o + (NB - _i) * P))
    _o = OFFS[-1][1]


def _copy_out(nc, tri, pss, i):
    """Scaled+biased PSUM->SBUF fp16 copy of row-block i. Whole blocks
    alternate between the ACT and DVE engines (even i -> ACT, odd i ->
    DVE): the two chains run in parallel and whole-block ops pay half
    the per-op DRAIN overhead of split halves."""
    o0, o1 = OFFS[i]
    w = o1 - o0
    if i % 2 == 0:
        nc.scalar.activation(
            out=tri[:, o0:o1],
            in_=pss[i][:, 0:w],
            func=mybir.ActivationFunctionType.Copy,
            scale=SCALE,
            bias=BIAS,
        )
    else:
        nc.vector.tensor_scalar(
            out=tri[:, o0:o1],
            in0=pss[i][:, 0:w],
            scalar1=SCALE,
            scalar2=BIAS,
            op0=mybir.AluOpType.mult,
            op1=mybir.AluOpType.add,
        )


def _split_free(width):
    """Split a moving free-dim into chunks <= 512 (one PSUM bank of fp32)."""
    out = []
    s = 0
    while s < width:
        w = min(512, width - s)
        out.append((s, s + w))
        s += w
    return out


def _build():
    nc = bacc.Bacc(num_devices=NCORES)

    x_sh = nc.dram_tensor(
        "x_shard", [NSHARD, D], mybir.dt.float32, kind="ExternalInput"
    )
    out_sh = nc.dram_tensor(
        "out_shard", [P, TRI_W], mybir.dt.float16, kind="ExternalOutput"
    )

    f32 = mybir.dt.float32
    bf16 = mybir.dt.bfloat16
    f16 = mybir.dt.float16
    f8 = mybir.dt.float8e4  # e4m3

    with tile.TileContext(nc) as tc:
        with (
            tc.tile_pool(name="xp", bufs=KT) as xpool,
            tc.tile_pool(name="bp", bufs=KBF + 1) as bpool,
            tc.tile_pool(name="f8p", bufs=NSUPER) as f8pool,
            tc.tile_pool(name="ps", bufs=1, space="PSUM") as pspool,
            tc.tile_pool(name="acc", bufs=1) as accpool,
        ):
            # HAM warmup tile: first thing in program order so the junk
            # matmuls fill the PE pipe while the first x tiles stream in
            warm = bpool.tile([P, 512], bf16, tag="warm", name="warm")
            nc.gpsimd.memset(warm[:], 0.0)

            # load pipeline: 16 single-tile HWDGE f32 DMAs alternating the
            # two physical rings (SP / ACT) — measured at the per-core HBM
            # roofline, and single-tile granularity keeps first-arrival
            # latency low so the PE never starves. (Pair-batched loads
            # were tried: fewer DMA boundaries, but each batch's first
            # tile arrives a full batch later -> 4us PE bubble. Slower.)
            # DVE casts: k0..1 -> bf16; k2..15 -> fp8 super-tile slices.
            xt_bf = []
            f8t = []
            for s in range(NSUPER):
                f8t.append(
                    f8pool.tile([P, 2, D], f8, tag="f8", name=f"f8_{s}")
                )
            # queue map: k0/k1 on the two HWDGE rings (lowest first-byte
            # latency), then round-robin [SWDGE, SP, ACT] so each queue's
            # per-DMA boundary bubble hides behind the other two streams
            load_eng = {}
            for k in range(KT):
                if k < 2:
                    load_eng[k] = nc.sync if k == 0 else nc.scalar
                else:
                    load_eng[k] = (nc.gpsimd, nc.sync, nc.scalar)[(k - 2) % 3]
            for k in range(KT):
                stage = xpool.tile([P, D], f32, tag="xs", name=f"xs{k}")
                load_eng[k].dma_start(out=stage[:], in_=x_sh[k * P : (k + 1) * P, :])
                if k < KBF:
                    xtile = bpool.tile([P, D], bf16, tag="xb", name=f"xb{k}")
                    nc.vector.tensor_copy(out=xtile[:], in_=stage[:])
                    xt_bf.append(xtile)
                else:
                    s, t = divmod(k - KBF, 2)
                    nc.vector.tensor_copy(out=f8t[s][:, t, :], in_=stage[:])

            tri = accpool.tile([P, TRI_W], f16)  # packed scaled triangle

            # psum accumulators, one per row-block; exactly 8 PSUM banks.
            # row-block i covers G[i-block, j-blocks j>=i] = cols 128*i..768
            pss = []
            for i in range(NB):
                pss.append(
                    pspool.tile([P, D - P * i], f32, tag=f"ps{i}", name=f"ps{i}")
                )

            # HAM warmup: junk matmuls on the zeroed tile keep the PE busy
            # (~3us cold) until the first real tile lands, so the HAM
            # activity window that un-throttles the PE clock fires early.
            # Junk goes to pss[0]; the real k=0 matmul has start=True
            # which resets it.
            for w in range(7):
                nc.tensor.matmul(
                    pss[0][:, 0:512],
                    lhsT=warm[:, 0:P],
                    rhs=warm[:],
                    start=True,
                    stop=True,
                )

            # per-tile matmul chunk list, ordered so consecutive matmuls
            # use different stationary weights (the 2nd chunk of i=0/i=1
            # is deferred) - lets LDWEIGHTS overlap the running matmul
            chunks = []  # (i, s0, s1)
            deferred = []
            for i in range(NB):
                sp = _split_free(D - P * i)
                chunks.append((i, sp[0][0], sp[0][1]))
                for s0, s1 in sp[1:]:
                    deferred.append((i, s0, s1))
            chunks[2:2] = deferred  # order: i0a, i1a, i0b, i1b, i2..i5

            # final super-tile: completion order (all chunks of block i
            # adjacent, big blocks first) so each row-block's copy-out +
            # store fires as early as possible and overlaps the rest
            last_chunks = []
            for i in range(NB):
                for s0, s1 in _split_free(D - P * i):
                    last_chunks.append((i, s0, s1))

            # store groups: fired after the last block in the group is
            # copied out; progressive so data drains while the PE still
            # works. The tiny final store goes on the ACT ring (which
            # just finished block 5's copy-out) so its completion
            # receipt starts as early as possible.
            store_groups = {
                0: ([0], nc.sync),
                2: ([1, 2], nc.sync),
                4: ([3, 4], nc.sync),
                5: ([5], nc.scalar),
            }

            # bf16 prologue: k = 0, 1 as single-k matmuls (PE starts on
            # the very first tile)
            for k in range(KBF):
                for i, s0, s1 in chunks:
                    c0 = P * i
                    nc.tensor.matmul(
                        pss[i][:, s0:s1],
                        lhsT=xt_bf[k][:, c0 : c0 + P],
                        rhs=xt_bf[k][:, c0 + s0 : c0 + s1],
                        start=(k == 0),
                        stop=False,
                    )

            # fp8 DoubleRow main loop: super-tile s = (k 2s+2, 2s+3),
            # 256-sample contraction per matmul at ~2x PE rate. The final
            # super runs its chunks in completion order; each finished
            # row-block's copy-out + store fires immediately and overlaps
            # the remaining matmuls.
            for s in range(NSUPER):
                last = s == NSUPER - 1
                ch = last_chunks if last else chunks
                for i, s0, s1 in ch:
                    c0 = P * i
                    nc.tensor.matmul(
                        pss[i][:, s0:s1],
                        lhsT=f8t[s][:, :, c0 : c0 + P],
                        rhs=f8t[s][:, :, c0 + s0 : c0 + s1],
                        start=False,
                        stop=last,
                        perf_mode=mybir.MatmulPerfMode.DoubleRow,
                    )
                    if last and s1 == D - P * i:
                        # block i finished: copy out while the PE works
                        # on the remaining blocks
                        _copy_out(nc, tri, pss, i)
                        if i in store_groups:
                            blocks, eng = store_groups[i]
                            o0 = OFFS[blocks[0]][0]
                            o1 = OFFS[blocks[-1]][1]
                            eng.dma_start(
                                out=out_sh[:, o0:o1], in_=tri[:, o0:o1]
                            )

    nc.finalize()  # Bacc: run reg-alloc + wait-legalization passes
    return nc


_NC_CACHE = None

# test-harness hooks (harness calls kernel() only; these stay defaults there)
RUN_KWARGS = {}
LAST_RESULTS = None


def _get_nc():
    global _NC_CACHE
    if _NC_CACHE is None:
        _NC_CACHE = _build()
    return _NC_CACHE


def kernel(x: np.ndarray) -> np.ndarray:
    global LAST_RESULTS
    x = np.ascontiguousarray(np.asarray(x, dtype=np.float32))
    assert x.shape == (NCORES, NSHARD, D)

    nc = _get_nc()
    in_maps = [{"x_shard": x[c]} for c in range(NCORES)]
    res = run_bass_kernel_spmd(
        nc, in_maps, core_ids=list(range(NCORES)), **RUN_KWARGS
    )
    LAST_RESULTS = res

    # gather/unshard: sum the per-core scaled partial triangles (fp16) in
    # fp32, then unpack the symmetric matrix
    packed = np.zeros((P, TRI_W), dtype=np.float32)
    for c in range(NCORES):
        packed += res.results[c]["out_shard"].astype(np.float32)
    packed = packed.reshape(P, NTRI, P).transpose(1, 0, 2)  # [21, 128, 128]

    out = np.empty((D, D), dtype=np.float32)
    for b, (i, j) in enumerate(TRI_BLOCKS):
        blk = packed[b]
        out[P * i : P * (i + 1), P * j : P * (j + 1)] = blk
        if j != i:
            out[P * j : P * (j + 1), P * i : P * (i + 1)] = blk.T
    return out


# revision 22
# speedup vs baseline: 1.1087x; 1.1087x over previous
"""Bass/Trainium2 kernel for nn_DecorrelationGradient.

Reference computation (KAPPA = 0.5):
    out = (1-k)*(gram - diag_ms) + k*(diag_ms - 1)
        = 0.5 * (X^T X / N) - 0.5          (diag terms cancel algebraically)

with X = x.reshape(N, d), N = 8*2048 = 16384, d = 768.

Strategy (data-parallel over the sample axis, 8 cores):
  - core c gets x[c] : [2048, 768] f32
  - per-core pipeline: HWDGE f32 loads (both rings, 16 single-tile DMAs:
    measured ~334-374 GB/s aggregate = the per-core HBM roofline) -> DVE
    casts -> PE accumulates the upper-triangle blocks of the partial
    Gram P_c = x_c^T x_c in PSUM (fp32)
  - PE dtype schedule: k-tiles 0-1 run as bf16 single-k matmuls (so the
    PE can start as soon as the FIRST tile lands); k-tiles 2-15 run as 7
    fp8e4m3 DoubleRow super-tiles (256-sample contraction, ~2x PE rate).
    This keeps the PE off the critical path even when the chip sits in
    the low power state (PE ~2.0 GHz instead of 2.4). fp8 quantization
    adds only ~2e-4 rel err: the Gram averages 16384 independent
    products, so per-product quantization noise shrinks ~sqrt(N).
  - junk matmuls on a zeroed tile bridge the PE from t=0 to the first
    real tile so the HAM clock-gate un-throttles before real work
  - fused scale+bias on the PSUM->SBUF copy (ACT/DVE split halves):
    t = P_c * (0.5/N) - 0.5/8, emitted in fp16 (halves store bytes,
    ~1e-4 rel err on the 8-partial sum)
  - the final super-tile's chunks run in completion order (big blocks
    first); finished row-block groups are stored immediately (3 stores
    on the idle SP ring) so stores overlap the remaining matmuls
  - each core outputs its scaled partial packed triangle [128, 2688]
    fp16; the host gather sums the 8 partials in fp32 (the affine above
    makes the sum equal 0.5*G/N - 0.5) and unpacks the symmetric matrix.
"""

import numpy as np

import concourse.bacc as bacc
import concourse.bass as bass  # noqa: F401
import concourse.tile as tile
from concourse import mybir
from concourse.bass_utils import run_bass_kernel_spmd

P = 128
D = 768
NSHARD = 2048          # samples per core
KT = NSHARD // P       # 16 k-tiles
KBF = 2                # leading k-tiles computed in bf16 (early PE start)
# k2..k15 run as 7 fp8 DoubleRow super-tiles (2 k-tiles each). A lone-k15
# finale was tried and is NOT better: it forces the (k13,k14) super to
# start only after k14's late cast, so the PE tail past the last cast
# grows from ~1.6us to ~2.1us.
NSUPER = (KT - KBF) // 2
NB = D // P            # 6 row/col blocks
NCORES = 8
NTOT = 8 * 2048
SCALE = 0.5 / NTOT     # 2**-15, exact
BIAS = -0.5 / NCORES   # -0.0625, exact in fp16; host sum of 8 -> -0.5

# packed upper-triangle blocks (i, j) with j >= i, row-major in i
TRI_BLOCKS = [(i, j) for i in range(NB) for j in range(i, NB)]
NTRI = len(TRI_BLOCKS)          # 21
TRI_W = NTRI * P                # 2688 packed columns

# packed column range of row-block i
OFFS = []
_o = 0
for _i in range(NB):
    OFFS.append((_o, _o + (NB - _i) * P))
    _o = OFFS[-1][1]


def _copy_out(nc, tri, pss, i):
    """Scaled+biased PSUM->SBUF fp16 copy of row-block i. Whole blocks
    alternate between the ACT and DVE engines (even i -> ACT, odd i ->
    DVE): the two chains run in parallel and whole-block ops pay half
    the per-op DRAIN overhead of split halves."""
    o0, o1 = OFFS[i]
    w = o1 - o0
    if i % 2 == 0:
        nc.scalar.activation(
            out=tri[:, o0:o1],
            in_=pss[i][:, 0:w],
            func=mybir.ActivationFunctionType.Copy,
            scale=SCALE,
            bias=BIAS,
        )
    else:
        nc.vector.tensor_scalar(
            out=tri[:, o0:o1],
            in0=pss[i][:, 0:w],
            scalar1=SCALE,
            scalar2=BIAS,
            op0=mybir.AluOpType.mult,
            op1=mybir.AluOpType.add,
        )


def _split_free(width):
    """Split a moving free-dim into chunks <= 512 (one PSUM bank of fp32)."""
    out = []
    s = 0
    while s < width:
        w = min(512, width - s)
        out.append((s, s + w))
        s += w
    return out


def _build():
    nc = bacc.Bacc(num_devices=NCORES)

    x_sh = nc.dram_tensor(
        "x_shard", [NSHARD, D], mybir.dt.float32, kind="ExternalInput"
    )
    out_sh = nc.dram_tensor(
        "out_shard", [P, TRI_W], mybir.dt.float16, kind="ExternalOutput"
    )

    f32 = mybir.dt.float32
    bf16 = mybir.dt.bfloat16
    f16 = mybir.dt.float16
    f8 = mybir.dt.float8e4  # e4m3

    with tile.TileContext(nc) as tc:
        with (
            tc.tile_pool(name="xp", bufs=KT) as xpool,
            tc.tile_pool(name="bp", bufs=KBF + 1) as bpool,
            tc.tile_pool(name="f8p", bufs=NSUPER) as f8pool,
            tc.tile_pool(name="ps", bufs=1, space="PSUM") as pspool,
            tc.tile_pool(name="acc", bufs=1) as accpool,
        ):
            # HAM warmup tile: first thing in program order so the junk
            # matmuls fill the PE pipe while the first x tiles stream in
            warm = bpool.tile([P, 512], bf16, tag="warm", name="warm")
            nc.gpsimd.memset(warm[:], 0.0)

            # load pipeline: 16 single-tile HWDGE f32 DMAs alternating the
            # two physical rings (SP / ACT) — measured at the per-core HBM
            # roofline, and single-tile granularity keeps first-arrival
            # latency low so the PE never starves. (Pair-batched loads
            # were tried: fewer DMA boundaries, but each batch's first
            # tile arrives a full batch later -> 4us PE bubble. Slower.)
            # DVE casts: k0..1 -> bf16; k2..15 -> fp8 super-tile slices.
            xt_bf = []
            f8t = []
            for s in range(NSUPER):
                f8t.append(
                    f8pool.tile([P, 2, D], f8, tag="f8", name=f"f8_{s}")
                )
            for k in range(KT):
                stage = xpool.tile([P, D], f32, tag="xs", name=f"xs{k}")
                dma_eng = nc.sync if k % 2 == 0 else nc.scalar
                dma_eng.dma_start(out=stage[:], in_=x_sh[k * P : (k + 1) * P, :])
                if k < KBF:
                    xtile = bpool.tile([P, D], bf16, tag="xb", name=f"xb{k}")
                    nc.vector.tensor_copy(out=xtile[:], in_=stage[:])
                    xt_bf.append(xtile)
                else:
                    s, t = divmod(k - KBF, 2)
                    nc.vector.tensor_copy(out=f8t[s][:, t, :], in_=stage[:])

            tri = accpool.tile([P, TRI_W], f16)  # packed scaled triangle

            # psum accumulators, one per row-block; exactly 8 PSUM banks.
            # row-block i covers G[i-block, j-blocks j>=i] = cols 128*i..768
            pss = []
            for i in range(NB):
                pss.append(
                    pspool.tile([P, D - P * i], f32, tag=f"ps{i}", name=f"ps{i}")
                )

            # HAM warmup: junk matmuls on the zeroed tile keep the PE busy
            # (~3us cold) until the first real tile lands, so the HAM
            # activity window that un-throttles the PE clock fires early.
            # Junk goes to pss[0]; the real k=0 matmul has start=True
            # which resets it.
            for w in range(7):
                nc.tensor.matmul(
                    pss[0][:, 0:512],
                    lhsT=warm[:, 0:P],
                    rhs=warm[:],
                    start=True,
                    stop=True,
                )

            # per-tile matmul chunk list, ordered so consecutive matmuls
            # use different stationary weights (the 2nd chunk of i=0/i=1
            # is deferred) - lets LDWEIGHTS overlap the running matmul
            chunks = []  # (i, s0, s1)
            deferred = []
            for i in range(NB):
                sp = _split_free(D - P * i)
                chunks.append((i, sp[0][0], sp[0][1]))
                for s0, s1 in sp[1:]:
                    deferred.append((i, s0, s1))
            chunks[2:2] = deferred  # order: i0a, i1a, i0b, i1b, i2..i5

            # final super-tile: completion order (all chunks of block i
            # adjacent, big blocks first) so each row-block's copy-out +
            # store fires as early as possible and overlaps the rest
            last_chunks = []
            for i in range(NB):
                for s0, s1 in _split_free(D - P * i):
                    last_chunks.append((i, s0, s1))

            # store groups: fired after the last block in the group is
            # copied out; progressive so data drains while the PE still
            # works. The tiny final store goes on the ACT ring (which
            # just finished block 5's copy-out) so its completion
            # receipt starts as early as possible.
            store_groups = {
                0: ([0], nc.sync),
                2: ([1, 2], nc.sync),
                4: ([3, 4], nc.sync),
                5: ([5], nc.scalar),
            }

            # bf16 prologue: k = 0, 1 as single-k matmuls (PE starts on
            # the very first tile)
            for k in range(KBF):
                for i, s0, s1 in chunks:
                    c0 = P * i
                    nc.tensor.matmul(
                        pss[i][:, s0:s1],
                        lhsT=xt_bf[k][:, c0 : c0 + P],
                        rhs=xt_bf[k][:, c0 + s0 : c0 + s1],
                        start=(k == 0),
                        stop=False,
                    )

            # fp8 DoubleRow main loop: super-tile s = (k 2s+2, 2s+3),
            # 256-sample contraction per matmul at ~2x PE rate. The final
            # super runs its chunks in completion order; each finished
            # row-block's copy-out + store fires immediately and overlaps
            # the remaining matmuls.
            for s in range(NSUPER):
                last = s == NSUPER - 1
                ch = last_chunks if last else chunks
                for i, s0, s1 in ch:
                    c0 = P * i
                    nc.tensor.matmul(
                        pss[i][:, s0:s1],
                        lhsT=f8t[s][:, :, c0 : c0 + P],
                        rhs=f8t[s][:, :, c0 + s0 : c0 + s1],
                        start=False,
                        stop=last,
                        perf_mode=mybir.MatmulPerfMode.DoubleRow,
                    )
                    if last and s1 == D - P * i:
                        # block i finished: copy out while the PE works
                        # on the remaining blocks
                        _copy_out(nc, tri, pss, i)
                        if i in store_groups:
                            blocks, eng = store_groups[i]
                            o0 = OFFS[blocks[0]][0]
                            o1 = OFFS[blocks[-1]][1]
                            eng.dma_start(
                                out=out_sh[:, o0:o1], in_=tri[:, o0:o1]
                            )

    nc.finalize()  # Bacc: run reg-alloc + wait-legalization passes
    return nc


_NC_CACHE = None

# test-harness hooks (harness calls kernel() only; these stay defaults there)
RUN_KWARGS = {}
LAST_RESULTS = None


def _get_nc():
    global _NC_CACHE
    if _NC_CACHE is None:
        _NC_CACHE = _build()
    return _NC_CACHE


def kernel(x: np.ndarray) -> np.ndarray:
    global LAST_RESULTS
    x = np.ascontiguousarray(np.asarray(x, dtype=np.float32))
    assert x.shape == (NCORES, NSHARD, D)

    nc = _get_nc()
    in_maps = [{"x_shard": x[c]} for c in range(NCORES)]
    res = run_bass_kernel_spmd(
        nc, in_maps, core_ids=list(range(NCORES)), **RUN_KWARGS
    )
    LAST_RESULTS = res

    # gather/unshard: sum the per-core scaled partial triangles (fp16) in
    # fp32, then unpack the symmetric matrix
    packed = np.zeros((P, TRI_W), dtype=np.float32)
    for c in range(NCORES):
        packed += res.results[c]["out_shard"].astype(np.float32)
    packed = packed.reshape(P, NTRI, P).transpose(1, 0, 2)  # [21, 128, 128]

    out = np.empty((D, D), dtype=np.float32)
    for b, (i, j) in enumerate(TRI_BLOCKS):
        blk = packed[b]
        out[P * i : P * (i + 1), P * j : P * (j + 1)] = blk
        if j != i:
            out[P * j : P * (j + 1), P * i : P * (i + 1)] = blk.T
    return out
